# revision 9
# baseline (speedup 1.0000x reference)
# LoRA-MoE QK kernel for 8x Trainium2 NeuronCores (Bass/Tile).
#
# Reference computation:
#   routing = softmax(mean(x[:, 611:-1, :]) @ router_W.T + router_b)   [B, E]
#   base    = x @ W.T + b
#   lora    = einsum('bsd,erd->bser', x, A) -> *B,routing -> [B,S,O] * 2.0
#   out     = base + lora
#
# Sharding: data-parallel over the 8192 tokens (1024/core; each core's tokens
# belong to exactly one batch; a batch spans cores {2b, 2b+1}).  Weights
# replicated.  Router computed on host (tiny [4,8] softmax).
#
# Precision strategy (rel-err budget 2e-2, bf16 floor is 2.0e-3):
#   The contraction D=4096 is split into 32 chunks of 128.  N8=10 chunks run
#   as 5 fp8e4m3 DoubleRow matmuls (256 contraction rows each, 2x PE rate);
#   the remaining 22 chunks run in bf16.  Host-measured rel err: 1.77e-2.
#   Scales: x*8 and W*512 (exact powers of 2) put operands in fp8 range; the
#   PSUM then holds 4096*(x@W.T + lora); bias is added as 4096*b on the DVE
#   and the host multiplies the final output by 2^-12.
#   LoRA t-phase uses the same fp8/bf16 split (error contribution ~0.2%).
#
# Startup pipelining: DMAs are issued in first-use order and the main loop's
# fp8 matmuls for the first 6 output groups are interleaved into the
# DMA-bound t-phase window, using all 8 PSUM banks (2 t-phase + 6 main).

import numpy as np
import ml_dtypes

BF16 = ml_dtypes.bfloat16
FP8 = ml_dtypes.float8_e4m3  # TRN variant: max normal 240

B_, S, D, O, E, R = 4, 2048, 4096, 4096, 8, 16
ER = E * R              # 128
TOK = B_ * S            # 8192
NCORES = 8
TPC = TOK // NCORES     # 1024 tokens per core
KT = D // 128           # 32 contraction chunks
NOB = O // 512          # 8 output-column panels
NTT = TPC // 128        # 8 token tiles per core
Q_LO, Q_HI = 611, 2047  # question tokens within each batch

N8 = 10                 # fp8 chunks (5 DoubleRow pairs)
NP8 = N8 // 2
NB = KT - N8            # bf16 chunks
SX, SW = 8.0, 512.0     # quantization scales (powers of 2)
OSCALE = np.float32(1.0 / (SX * SW))

_CACHE: dict = {}
LAST_RESULTS = None
TRACE = False


def _build_nc(num_devices=NCORES):
    import concourse.bacc as bacc
    import concourse.mybir as mybir
    from concourse import tile

    fp32 = mybir.dt.float32
    bf16 = mybir.dt.bfloat16
    fp8 = mybir.dt.float8e4
    DR = mybir.MatmulPerfMode.DoubleRow

    nc = bacc.Bacc(
        "TRN2",
        target_bir_lowering=False,
        debug=False,
        num_devices=num_devices,
    )

    x8 = nc.dram_tensor("x8", [128, N8, TPC], fp8, kind="ExternalInput")
    xb = nc.dram_tensor("xb", [128, NB, TPC], bf16, kind="ExternalInput")
    w8 = nc.dram_tensor("w8", [NOB * 128, N8, 512], fp8, kind="ExternalInput")
    wb = nc.dram_tensor("wb", [NOB * 128, NB, 512], bf16, kind="ExternalInput")
    af8 = nc.dram_tensor("af8", [128, N8, ER], fp8, kind="ExternalInput")
    afb = nc.dram_tensor("afb", [128, NB, ER], bf16, kind="ExternalInput")
    bfT = nc.dram_tensor("bfT", [ER, O], bf16, kind="ExternalInput")
    biasrep = nc.dram_tensor("biasrep", [128, O], bf16, kind="ExternalInput")
    svec = nc.dram_tensor("svec", [128, 1], fp32, kind="ExternalInput")
    out = nc.dram_tensor("out", [TPC, O], bf16, kind="ExternalOutput")

    with tile.TileContext(nc) as tc:
        with (
            tc.tile_pool(name="const", bufs=1) as const,
            tc.tile_pool(name="w", bufs=2) as wpool,
            tc.tile_pool(name="ot", bufs=4) as otpool,
            tc.tile_pool(name="ps", bufs=8, space="PSUM") as ps_pool,
        ):
            # ---- resident SBUF tensors ----
            x8_sb = const.tile([128, N8, TPC], fp8)
            xb_sb = const.tile([128, NB, TPC], bf16)
            af8_sb = const.tile([128, N8, ER], fp8)
            afb_sb = const.tile([128, NB, ER], bf16)
            bfT_sb = const.tile([128, O], bf16)
            biasrep_sb = const.tile([128, O], bf16)
            svec_sb = const.tile([128, 1], fp32)
            u_sb = const.tile([128, TPC], bf16)    # [er, t]

            dmy_sb = const.tile([128, 512], bf16)

            # ---- DMAs spread over four engine queues (a single queue
            # serializes at ~240 GB/s):
            #   sync:   x8 + even xb chunks + out    scalar: W panels + bfT/bias
            #   gpsimd: consts + odd xb chunks
            nc.vector.memset(dmy_sb[:], 0)
            nc.sync.dma_start(x8_sb[:], x8[:])

            w8_tiles = [None] * NOB
            wb_tiles = [None] * NOB

            def load_w8(ob):
                t = wpool.tile([128, N8, 512], fp8, tag="w8", name=f"w8_{ob}")
                nc.scalar.dma_start(t[:], w8[ob * 128:(ob + 1) * 128, :, :])
                w8_tiles[ob] = t

            def load_wb(ob):
                t = wpool.tile([128, NB, 512], bf16, tag="wb", name=f"wb_{ob}")
                nc.scalar.dma_start(t[:], wb[ob * 128:(ob + 1) * 128, :, :])
                wb_tiles[ob] = t

            load_w8(0)
            nc.gpsimd.dma_start(af8_sb[:], af8[:])
            nc.gpsimd.dma_start(svec_sb[:], svec[:])
            nc.gpsimd.dma_start(afb_sb[:], afb[:])
            # xb in 4 big-line quarter DMAs, alternating the two free queues
            # (many small 2KB-line DMAs starve against the W panel stream)
            for i, (lo, hi) in enumerate([(0, 6), (6, 11), (11, 17), (17, NB)]):
                eng = nc.sync if i % 2 == 0 else nc.gpsimd
                eng.dma_start(xb_sb[:, lo:hi, :], xb[:, lo:hi, :])
            load_wb(0)
            nc.scalar.dma_start(bfT_sb[:], bfT[:])
            for kk in range(4):
                nc.scalar.dma_start(
                    biasrep_sb[:, kk * 1024:(kk + 1) * 1024],
                    biasrep[:, kk * 1024:(kk + 1) * 1024],
                )

            # ---- warm-up: bridge the ~7us launch prolog + first DMAs so
            # the HAM un-throttles before the real MM stream starts ----
            NWARM = 14
            dmy_ps = ps_pool.tile([128, 512], fp32, tag="ps", name="dmy_ps")
            for i in range(NWARM):
                nc.tensor.matmul(
                    dmy_ps[:],
                    dmy_sb[:, 0:128],
                    dmy_sb[:],
                    start=(i == 0),
                    stop=(i == NWARM - 1),
                )

            # ---- t-phase fp8: t = Af8 @ x8 -> psum [er, t] ----
            pt_tiles = []
            for tb in range(2):
                pt = ps_pool.tile([128, 512], fp32, tag="ps", name=f"pt{tb}")
                pt_tiles.append(pt)
                for kt in range(NP8):
                    nc.tensor.matmul(
                        pt[:],
                        af8_sb[:, 2 * kt:2 * kt + 2, :],
                        x8_sb[:, 2 * kt:2 * kt + 2, tb * 512:tb * 512 + 512],
                        start=(kt == 0),
                        stop=False,
                        perf_mode=DR,
                    )

            # ---- interleaved: main-loop fp8 parts for ob=0, tt=0..5 ----
            po_tiles = [None] * NTT

            def main_fp8(ob, tt):
                po = ps_pool.tile([128, 512], fp32, tag="ps", name=f"po_{ob}_{tt}")
                po_tiles[tt] = po
                for kt in range(NP8):
                    nc.tensor.matmul(
                        po[:],
                        x8_sb[:, 2 * kt:2 * kt + 2, tt * 128:tt * 128 + 128],
                        w8_tiles[ob][:, 2 * kt:2 * kt + 2, :],
                        start=(kt == 0),
                        stop=False,
                        perf_mode=DR,
                    )

            def main_rest(ob, tt):
                po = po_tiles[tt]
                for kb in range(NB):
                    nc.tensor.matmul(
                        po[:],
                        xb_sb[:, kb:kb + 1, tt * 128:tt * 128 + 128],
                        wb_tiles[ob][:, kb:kb + 1, :],
                        start=False,
                        stop=False,
                    )
                nc.tensor.matmul(
                    po[:],
                    u_sb[:, tt * 128:(tt + 1) * 128],
                    bfT_sb[:, ob * 512:(ob + 1) * 512],
                    start=False,
                    stop=True,
                )
                ot = otpool.tile([128, 512], bf16, tag="ot", name=f"ot_{ob}_{tt}")
                nc.vector.tensor_add(
                    ot[:], po[:], biasrep_sb[:, ob * 512:(ob + 1) * 512]
                )
                nc.sync.dma_start(
                    out[tt * 128:(tt + 1) * 128, ob * 512:(ob + 1) * 512],
                    ot[:],
                )

            for tt in range(6):
                main_fp8(0, tt)

            # ---- t-phase bf16 (per-chunk, both halves), then u = t * routing ----
            for kb in range(NB):
                for tb in range(2):
                    nc.tensor.matmul(
                        pt_tiles[tb][:],
                        afb_sb[:, kb:kb + 1, :],
                        xb_sb[:, kb:kb + 1, tb * 512:tb * 512 + 512],
                        start=False,
                        stop=(kb == NB - 1),
                    )
            for tb in range(2):
                nc.vector.tensor_scalar_mul(
                    u_sb[:, tb * 512:(tb + 1) * 512],
                    pt_tiles[tb][:],
                    svec_sb[:, 0:1],
                )

            # ---- main loop ----
            for ob in range(NOB):
                if w8_tiles[ob] is None:
                    load_w8(ob)
                if wb_tiles[ob] is None:
                    load_wb(ob)
                if ob == 0:
                    for tt in range(6):
                        main_rest(0, tt)
                    for tt in range(6, NTT):
                        main_fp8(0, tt)
                        main_rest(0, tt)
                else:
                    for tt in range(NTT):
                        main_fp8(ob, tt)
                        main_rest(ob, tt)

    nc.compile()
    return nc


def _q8(v):
    return np.clip(v, -240.0, 240.0).astype(FP8)


def _host_prep(x, W, b, A, B, router_W, router_b):
    f32 = np.float32
    XT = np.ascontiguousarray(x, dtype=f32).reshape(TOK, D).T  # [D, TOK]
    # fp8 rows [0 : N8*128), bf16 rows [N8*128 : D); chunk c row = c*128+p
    x8_all = np.ascontiguousarray(
        _q8(XT[:N8 * 128] * SX).reshape(N8, 128, TOK).transpose(1, 0, 2)
    )
    xb_all = np.ascontiguousarray(
        (XT[N8 * 128:] * SX).astype(BF16).reshape(NB, 128, TOK).transpose(1, 0, 2)
    )

    WT = np.asarray(W, f32).T  # [D, O]
    w8_h = np.ascontiguousarray(
        _q8(WT[:N8 * 128] * SW)
        .reshape(N8, 128, NOB, 512).transpose(2, 1, 0, 3)
    ).reshape(NOB * 128, N8, 512)
    wb_h = np.ascontiguousarray(
        (WT[N8 * 128:] * SW).astype(BF16)
        .reshape(NB, 128, NOB, 512).transpose(2, 1, 0, 3)
    ).reshape(NOB * 128, NB, 512)

    AfT = np.asarray(A, f32).reshape(ER, D).T  # [D, ER]
    af8_h = np.ascontiguousarray(
        _q8(AfT[:N8 * 128] * SW).reshape(N8, 128, ER).transpose(1, 0, 2)
    )
    afb_h = np.ascontiguousarray(
        (AfT[N8 * 128:] * SW).astype(BF16).reshape(NB, 128, ER).transpose(1, 0, 2)
    )

    bfT_h = (2.0 * np.transpose(B, (0, 2, 1)).reshape(ER, O)).astype(BF16)
    bias_h = np.ascontiguousarray(
        np.broadcast_to((np.asarray(b, f32) * (SX * SW)).astype(BF16)[None, :], (128, O))
    )

    # router on host (numpy, float64 — exact vs device quantization noise)
    xq = np.asarray(x, np.float64)[:, Q_LO:Q_HI, :]
    q = xq.mean(axis=1)
    logits = q @ np.asarray(router_W, np.float64).T + np.asarray(router_b, np.float64)
    ex = np.exp(logits - logits.max(-1, keepdims=True))
    routing = ex / ex.sum(-1, keepdims=True)          # [B, E]

    in_maps = []
    for c in range(NCORES):
        sv = np.repeat(routing[c // 2].astype(f32), R).reshape(128, 1)
        in_maps.append({
            "x8": np.ascontiguousarray(x8_all[:, :, c * TPC:(c + 1) * TPC]),
            "xb": np.ascontiguousarray(xb_all[:, :, c * TPC:(c + 1) * TPC]),
            "w8": w8_h,
            "wb": wb_h,
            "af8": af8_h,
            "afb": afb_h,
            "bfT": bfT_h,
            "biasrep": bias_h,
            "svec": np.ascontiguousarray(sv),
        })
    return in_maps


def kernel(x, W, b, A, B, router_W, router_b):
    global LAST_RESULTS
    from concourse.bass_utils import run_bass_kernel_spmd

    if "nc" not in _CACHE:
        _CACHE["nc"] = _build_nc()
    nc = _CACHE["nc"]

    in_maps = _host_prep(x, W, b, A, B, router_W, router_b)

    kwargs = {}
    if TRACE:
        kwargs.update(trace=True, trace_cores=list(range(NCORES)))
    res = run_bass_kernel_spmd(nc, in_maps, core_ids=list(range(NCORES)), **kwargs)
    LAST_RESULTS = res

    shards = [
        np.asarray(res.results[c]["out"]).astype(np.float32) for c in range(NCORES)
    ]
    full = np.concatenate(shards, axis=0) * OSCALE
    return full.reshape(B_, S, O).astype(np.float32)


# revision 10
# speedup vs baseline: 1.0396x; 1.0396x over previous
# LoRA-MoE QK kernel for 8x Trainium2 NeuronCores (Bass/Tile).
#
# Reference computation:
#   routing = softmax(mean(x[:, 611:-1, :]) @ router_W.T + router_b)   [B, E]
#   base    = x @ W.T + b
#   lora    = einsum('bsd,erd->bser', x, A) -> *B,routing -> [B,S,O] * 2.0
#   out     = base + lora
#
# Sharding: data-parallel over the 8192 tokens (1024/core; each core's tokens
# belong to exactly one batch; a batch spans cores {2b, 2b+1}).  Weights
# replicated.  Router computed on host (tiny [4,8] softmax).
#
# Precision strategy (rel-err budget 2e-2, bf16 floor is 2.0e-3):
#   The contraction D=4096 is split into 32 chunks of 128.  N8=10 chunks run
#   as 5 fp8e4m3 DoubleRow matmuls (256 contraction rows each, 2x PE rate);
#   the remaining 22 chunks run in bf16.  Host-measured rel err: 1.77e-2.
#   Scales: x*8 and W*512 (exact powers of 2) put operands in fp8 range; the
#   PSUM then holds 4096*(x@W.T + lora); bias is added as 4096*b on the DVE
#   and the host multiplies the final output by 2^-12.
#   LoRA t-phase uses the same fp8/bf16 split (error contribution ~0.2%).
#
# Startup pipelining: DMAs are issued in first-use order and the main loop's
# fp8 matmuls for the first 6 output groups are interleaved into the
# DMA-bound t-phase window, using all 8 PSUM banks (2 t-phase + 6 main).

import numpy as np
import ml_dtypes

BF16 = ml_dtypes.bfloat16
FP8 = ml_dtypes.float8_e4m3  # TRN variant: max normal 240

B_, S, D, O, E, R = 4, 2048, 4096, 4096, 8, 16
ER = E * R              # 128
TOK = B_ * S            # 8192
NCORES = 8
TPC = TOK // NCORES     # 1024 tokens per core
KT = D // 128           # 32 contraction chunks
NOB = O // 512          # 8 output-column panels
NTT = TPC // 128        # 8 token tiles per core
Q_LO, Q_HI = 611, 2047  # question tokens within each batch

N8 = 12                 # fp8 chunks (6 DoubleRow pairs)
NP8 = N8 // 2
NB = KT - N8            # bf16 chunks
SX, SW = 8.0, 512.0     # quantization scales (powers of 2)
OSCALE = np.float32(1.0 / (SX * SW))

_CACHE: dict = {}
LAST_RESULTS = None
TRACE = False


def _build_nc(num_devices=NCORES):
    import concourse.bacc as bacc
    import concourse.mybir as mybir
    from concourse import tile

    fp32 = mybir.dt.float32
    bf16 = mybir.dt.bfloat16
    fp8 = mybir.dt.float8e4
    DR = mybir.MatmulPerfMode.DoubleRow

    nc = bacc.Bacc(
        "TRN2",
        target_bir_lowering=False,
        debug=False,
        num_devices=num_devices,
    )

    x8 = nc.dram_tensor("x8", [128, N8, TPC], fp8, kind="ExternalInput")
    xb = nc.dram_tensor("xb", [128, NB, TPC], bf16, kind="ExternalInput")
    w8 = nc.dram_tensor("w8", [NOB * 128, N8, 512], fp8, kind="ExternalInput")
    wb = nc.dram_tensor("wb", [NOB * 128, NB, 512], bf16, kind="ExternalInput")
    af8 = nc.dram_tensor("af8", [128, N8, ER], fp8, kind="ExternalInput")
    afb = nc.dram_tensor("afb", [128, NB, ER], bf16, kind="ExternalInput")
    bfT = nc.dram_tensor("bfT", [ER, O], bf16, kind="ExternalInput")
    biasrep = nc.dram_tensor("biasrep", [128, O], bf16, kind="ExternalInput")
    svec = nc.dram_tensor("svec", [128, 1], fp32, kind="ExternalInput")
    out = nc.dram_tensor("out", [TPC, O], bf16, kind="ExternalOutput")

    with tile.TileContext(nc) as tc:
        with (
            tc.tile_pool(name="const", bufs=1) as const,
            tc.tile_pool(name="w", bufs=2) as wpool,
            tc.tile_pool(name="ot", bufs=4) as otpool,
            tc.tile_pool(name="ps", bufs=8, space="PSUM") as ps_pool,
        ):
            # ---- resident SBUF tensors ----
            x8_sb = const.tile([128, N8, TPC], fp8)
            xb_sb = const.tile([128, NB, TPC], bf16)
            af8_sb = const.tile([128, N8, ER], fp8)
            afb_sb = const.tile([128, NB, ER], bf16)
            bfT_sb = const.tile([128, O], bf16)
            biasrep_sb = const.tile([128, O], bf16)
            svec_sb = const.tile([128, 1], fp32)
            u_sb = const.tile([128, TPC], bf16)    # [er, t]

            dmy_sb = const.tile([128, 512], bf16)

            # ---- DMAs spread over four engine queues (a single queue
            # serializes at ~240 GB/s):
            #   sync:   x8 + even xb chunks + out    scalar: W panels + bfT/bias
            #   gpsimd: consts + odd xb chunks
            nc.vector.memset(dmy_sb[:], 0)
            nc.sync.dma_start(x8_sb[:], x8[:])

            w8_tiles = [None] * NOB
            wb_tiles = [None] * NOB

            def load_w8(ob):
                t = wpool.tile([128, N8, 512], fp8, tag="w8", name=f"w8_{ob}")
                nc.scalar.dma_start(t[:], w8[ob * 128:(ob + 1) * 128, :, :])
                w8_tiles[ob] = t

            def load_wb(ob):
                t = wpool.tile([128, NB, 512], bf16, tag="wb", name=f"wb_{ob}")
                nc.scalar.dma_start(t[:], wb[ob * 128:(ob + 1) * 128, :, :])
                wb_tiles[ob] = t

            load_w8(0)
            nc.gpsimd.dma_start(af8_sb[:], af8[:])
            nc.gpsimd.dma_start(svec_sb[:], svec[:])
            nc.gpsimd.dma_start(afb_sb[:], afb[:])
            # xb per-chunk, alternating the two free queues
            for kb in range(NB):
                eng = nc.sync if kb % 2 == 0 else nc.gpsimd
                eng.dma_start(xb_sb[:, kb:kb + 1, :], xb[:, kb:kb + 1, :])
            load_wb(0)
            nc.scalar.dma_start(bfT_sb[:], bfT[:])
            for kk in range(4):
                nc.scalar.dma_start(
                    biasrep_sb[:, kk * 1024:(kk + 1) * 1024],
                    biasrep[:, kk * 1024:(kk + 1) * 1024],
                )

            # ---- warm-up: bridge the ~7us launch prolog + first DMAs so
            # the HAM un-throttles before the real MM stream starts ----
            NWARM = 14
            dmy_ps = ps_pool.tile([128, 512], fp32, tag="ps", name="dmy_ps")
            for i in range(NWARM):
                nc.tensor.matmul(
                    dmy_ps[:],
                    dmy_sb[:, 0:128],
                    dmy_sb[:],
                    start=(i == 0),
                    stop=(i == NWARM - 1),
                )

            # ---- t-phase fp8: t = Af8 @ x8 -> psum [er, t] ----
            pt_tiles = []
            for tb in range(2):
                pt = ps_pool.tile([128, 512], fp32, tag="ps", name=f"pt{tb}")
                pt_tiles.append(pt)
                for kt in range(NP8):
                    nc.tensor.matmul(
                        pt[:],
                        af8_sb[:, 2 * kt:2 * kt + 2, :],
                        x8_sb[:, 2 * kt:2 * kt + 2, tb * 512:tb * 512 + 512],
                        start=(kt == 0),
                        stop=False,
                        perf_mode=DR,
                    )

            # ---- interleaved: main-loop fp8 parts for ob=0, tt=0..5 ----
            po_tiles = [None] * NTT

            def main_fp8(ob, tt):
                po = ps_pool.tile([128, 512], fp32, tag="ps", name=f"po_{ob}_{tt}")
                po_tiles[tt] = po
                for kt in range(NP8):
                    nc.tensor.matmul(
                        po[:],
                        x8_sb[:, 2 * kt:2 * kt + 2, tt * 128:tt * 128 + 128],
                        w8_tiles[ob][:, 2 * kt:2 * kt + 2, :],
                        start=(kt == 0),
                        stop=False,
                        perf_mode=DR,
                    )

            def main_rest(ob, tt):
                po = po_tiles[tt]
                for kb in range(NB):
                    nc.tensor.matmul(
                        po[:],
                        xb_sb[:, kb:kb + 1, tt * 128:tt * 128 + 128],
                        wb_tiles[ob][:, kb:kb + 1, :],
                        start=False,
                        stop=False,
                    )
                nc.tensor.matmul(
                    po[:],
                    u_sb[:, tt * 128:(tt + 1) * 128],
                    bfT_sb[:, ob * 512:(ob + 1) * 512],
                    start=False,
                    stop=True,
                )
                ot = otpool.tile([128, 512], bf16, tag="ot", name=f"ot_{ob}_{tt}")
                nc.vector.tensor_add(
                    ot[:], po[:], biasrep_sb[:, ob * 512:(ob + 1) * 512]
                )
                nc.sync.dma_start(
                    out[tt * 128:(tt + 1) * 128, ob * 512:(ob + 1) * 512],
                    ot[:],
                )

            for tt in range(6):
                main_fp8(0, tt)

            # ---- t-phase bf16 (per-chunk, both halves), then u = t * routing ----
            for kb in range(NB):
                for tb in range(2):
                    nc.tensor.matmul(
                        pt_tiles[tb][:],
                        afb_sb[:, kb:kb + 1, :],
                        xb_sb[:, kb:kb + 1, tb * 512:tb * 512 + 512],
                        start=False,
                        stop=(kb == NB - 1),
                    )
            for tb in range(2):
                nc.vector.tensor_scalar_mul(
                    u_sb[:, tb * 512:(tb + 1) * 512],
                    pt_tiles[tb][:],
                    svec_sb[:, 0:1],
                )

            # ---- main loop ----
            for ob in range(NOB):
                if w8_tiles[ob] is None:
                    load_w8(ob)
                if wb_tiles[ob] is None:
                    load_wb(ob)
                if ob == 0:
                    for tt in range(6):
                        main_rest(0, tt)
                    for tt in range(6, NTT):
                        main_fp8(0, tt)
                        main_rest(0, tt)
                else:
                    for tt in range(NTT):
                        main_fp8(ob, tt)
                        main_rest(ob, tt)

    nc.compile()
    return nc


def _q8(v):
    return np.clip(v, -240.0, 240.0).astype(FP8)


def _host_prep(x, W, b, A, B, router_W, router_b):
    f32 = np.float32
    XT = np.ascontiguousarray(x, dtype=f32).reshape(TOK, D).T  # [D, TOK]
    # fp8 rows [0 : N8*128), bf16 rows [N8*128 : D); chunk c row = c*128+p
    x8_all = np.ascontiguousarray(
        _q8(XT[:N8 * 128] * SX).reshape(N8, 128, TOK).transpose(1, 0, 2)
    )
    xb_all = np.ascontiguousarray(
        (XT[N8 * 128:] * SX).astype(BF16).reshape(NB, 128, TOK).transpose(1, 0, 2)
    )

    WT = np.asarray(W, f32).T  # [D, O]
    w8_h = np.ascontiguousarray(
        _q8(WT[:N8 * 128] * SW)
        .reshape(N8, 128, NOB, 512).transpose(2, 1, 0, 3)
    ).reshape(NOB * 128, N8, 512)
    wb_h = np.ascontiguousarray(
        (WT[N8 * 128:] * SW).astype(BF16)
        .reshape(NB, 128, NOB, 512).transpose(2, 1, 0, 3)
    ).reshape(NOB * 128, NB, 512)

    AfT = np.asarray(A, f32).reshape(ER, D).T  # [D, ER]
    af8_h = np.ascontiguousarray(
        _q8(AfT[:N8 * 128] * SW).reshape(N8, 128, ER).transpose(1, 0, 2)
    )
    afb_h = np.ascontiguousarray(
        (AfT[N8 * 128:] * SW).astype(BF16).reshape(NB, 128, ER).transpose(1, 0, 2)
    )

    bfT_h = (2.0 * np.transpose(B, (0, 2, 1)).reshape(ER, O)).astype(BF16)
    bias_h = np.ascontiguousarray(
        np.broadcast_to((np.asarray(b, f32) * (SX * SW)).astype(BF16)[None, :], (128, O))
    )

    # router on host (numpy, float64 — exact vs device quantization noise)
    xq = np.asarray(x, np.float64)[:, Q_LO:Q_HI, :]
    q = xq.mean(axis=1)
    logits = q @ np.asarray(router_W, np.float64).T + np.asarray(router_b, np.float64)
    ex = np.exp(logits - logits.max(-1, keepdims=True))
    routing = ex / ex.sum(-1, keepdims=True)          # [B, E]

    in_maps = []
    for c in range(NCORES):
        sv = np.repeat(routing[c // 2].astype(f32), R).reshape(128, 1)
        in_maps.append({
            "x8": np.ascontiguousarray(x8_all[:, :, c * TPC:(c + 1) * TPC]),
            "xb": np.ascontiguousarray(xb_all[:, :, c * TPC:(c + 1) * TPC]),
            "w8": w8_h,
            "wb": wb_h,
            "af8": af8_h,
            "afb": afb_h,
            "bfT": bfT_h,
            "biasrep": bias_h,
            "svec": np.ascontiguousarray(sv),
        })
    return in_maps


def kernel(x, W, b, A, B, router_W, router_b):
    global LAST_RESULTS
    from concourse.bass_utils import run_bass_kernel_spmd

    if "nc" not in _CACHE:
        _CACHE["nc"] = _build_nc()
    nc = _CACHE["nc"]

    in_maps = _host_prep(x, W, b, A, B, router_W, router_b)

    kwargs = {}
    if TRACE:
        kwargs.update(trace=True, trace_cores=list(range(NCORES)))
    res = run_bass_kernel_spmd(nc, in_maps, core_ids=list(range(NCORES)), **kwargs)
    LAST_RESULTS = res

    shards = [
        np.asarray(res.results[c]["out"]).astype(np.float32) for c in range(NCORES)
    ]
    full = np.concatenate(shards, axis=0) * OSCALE
    return full.reshape(B_, S, O).astype(np.float32)


# revision 11
# speedup vs baseline: 1.0453x; 1.0055x over previous
# LoRA-MoE QK kernel for 8x Trainium2 NeuronCores (Bass/Tile).
#
# Reference computation:
#   routing = softmax(mean(x[:, 611:-1, :]) @ router_W.T + router_b)   [B, E]
#   base    = x @ W.T + b
#   lora    = einsum('bsd,erd->bser', x, A) -> *B,routing -> [B,S,O] * 2.0
#   out     = base + lora
#
# Sharding: data-parallel over the 8192 tokens (1024/core; each core's tokens
# belong to exactly one batch; a batch spans cores {2b, 2b+1}).  Weights
# replicated.  Router computed on host (tiny [4,8] softmax).
#
# Precision strategy (rel-err budget 2e-2, bf16 floor is 2.0e-3):
#   The contraction D=4096 is split into 32 chunks of 128.  N8=10 chunks run
#   as 5 fp8e4m3 DoubleRow matmuls (256 contraction rows each, 2x PE rate);
#   the remaining 22 chunks run in bf16.  Host-measured rel err: 1.77e-2.
#   Scales: x*8 and W*512 (exact powers of 2) put operands in fp8 range; the
#   PSUM then holds 4096*(x@W.T + lora); bias is added as 4096*b on the DVE
#   and the host multiplies the final output by 2^-12.
#   LoRA t-phase uses the same fp8/bf16 split (error contribution ~0.2%).
#
# Startup pipelining: DMAs are issued in first-use order and the main loop's
# fp8 matmuls for the first 6 output groups are interleaved into the
# DMA-bound t-phase window, using all 8 PSUM banks (2 t-phase + 6 main).

import numpy as np
import ml_dtypes

BF16 = ml_dtypes.bfloat16
FP8 = ml_dtypes.float8_e4m3  # TRN variant: max normal 240

B_, S, D, O, E, R = 4, 2048, 4096, 4096, 8, 16
ER = E * R              # 128
TOK = B_ * S            # 8192
NCORES = 8
TPC = TOK // NCORES     # 1024 tokens per core
KT = D // 128           # 32 contraction chunks
NOB = O // 512          # 8 output-column panels
NTT = TPC // 128        # 8 token tiles per core
Q_LO, Q_HI = 611, 2047  # question tokens within each batch

N8 = 12                 # fp8 chunks (6 DoubleRow pairs)
NP8 = N8 // 2
NB = KT - N8            # bf16 chunks
SX, SW = 8.0, 512.0     # quantization scales (powers of 2)
OSCALE = np.float32(1.0 / (SX * SW))

_CACHE: dict = {}
LAST_RESULTS = None
TRACE = False


def _build_nc(num_devices=NCORES):
    import concourse.bacc as bacc
    import concourse.mybir as mybir
    from concourse import tile

    fp32 = mybir.dt.float32
    bf16 = mybir.dt.bfloat16
    fp8 = mybir.dt.float8e4
    DR = mybir.MatmulPerfMode.DoubleRow

    nc = bacc.Bacc(
        "TRN2",
        target_bir_lowering=False,
        debug=False,
        num_devices=num_devices,
    )

    x8 = nc.dram_tensor("x8", [128, N8, TPC], fp8, kind="ExternalInput")
    xb = nc.dram_tensor("xb", [128, NB, TPC], bf16, kind="ExternalInput")
    w8 = nc.dram_tensor("w8", [NOB * 128, N8, 512], fp8, kind="ExternalInput")
    wb = nc.dram_tensor("wb", [NOB * 128, NB, 512], bf16, kind="ExternalInput")
    af8 = nc.dram_tensor("af8", [128, N8, ER], fp8, kind="ExternalInput")
    afb = nc.dram_tensor("afb", [128, NB, ER], bf16, kind="ExternalInput")
    bfT = nc.dram_tensor("bfT", [ER, O], bf16, kind="ExternalInput")
    biasrep = nc.dram_tensor("biasrep", [128, O], bf16, kind="ExternalInput")
    svec = nc.dram_tensor("svec", [128, 1], fp32, kind="ExternalInput")
    out = nc.dram_tensor("out", [TPC, O], bf16, kind="ExternalOutput")

    with tile.TileContext(nc) as tc:
        with (
            tc.tile_pool(name="const", bufs=1) as const,
            tc.tile_pool(name="w", bufs=2) as wpool,
            tc.tile_pool(name="ot", bufs=4) as otpool,
            tc.tile_pool(name="ps", bufs=8, space="PSUM") as ps_pool,
        ):
            # ---- resident SBUF tensors ----
            x8_sb = const.tile([128, N8, TPC], fp8)
            xb_sb = const.tile([128, NB, TPC], bf16)
            af8_sb = const.tile([128, N8, ER], fp8)
            afb_sb = const.tile([128, NB, ER], bf16)
            bfT_sb = const.tile([128, O], bf16)
            biasrep_sb = const.tile([128, O], bf16)
            svec_sb = const.tile([128, 1], fp32)
            u_sb = const.tile([128, TPC], bf16)    # [er, t]

            dmy_sb = const.tile([128, 512], bf16)

            # ---- DMAs spread over four engine queues (a single queue
            # serializes at ~240 GB/s):
            #   sync:   x8 + even xb chunks + out    scalar: W panels + bfT/bias
            #   gpsimd: consts + odd xb chunks
            nc.vector.memset(dmy_sb[:], 0)
            nc.sync.dma_start(x8_sb[:, 0:N8 // 2, :], x8[:, 0:N8 // 2, :])
            nc.gpsimd.dma_start(x8_sb[:, N8 // 2:, :], x8[:, N8 // 2:, :])

            w8_tiles = [None] * NOB
            wb_tiles = [None] * NOB

            def load_w8(ob):
                t = wpool.tile([128, N8, 512], fp8, tag="w8", name=f"w8_{ob}")
                nc.scalar.dma_start(t[:], w8[ob * 128:(ob + 1) * 128, :, :])
                w8_tiles[ob] = t

            def load_wb(ob):
                t = wpool.tile([128, NB, 512], bf16, tag="wb", name=f"wb_{ob}")
                nc.scalar.dma_start(t[:], wb[ob * 128:(ob + 1) * 128, :, :])
                wb_tiles[ob] = t

            load_w8(0)
            nc.gpsimd.dma_start(af8_sb[:], af8[:])
            nc.gpsimd.dma_start(svec_sb[:], svec[:])
            nc.gpsimd.dma_start(afb_sb[:], afb[:])
            # xb per-chunk, alternating the two free queues
            for kb in range(NB):
                eng = nc.sync if kb % 2 == 0 else nc.gpsimd
                eng.dma_start(xb_sb[:, kb:kb + 1, :], xb[:, kb:kb + 1, :])
            load_wb(0)
            nc.scalar.dma_start(bfT_sb[:], bfT[:])
            for kk in range(4):
                nc.scalar.dma_start(
                    biasrep_sb[:, kk * 1024:(kk + 1) * 1024],
                    biasrep[:, kk * 1024:(kk + 1) * 1024],
                )

            # ---- warm-up: bridge the ~7us launch prolog + first DMAs so
            # the HAM un-throttles before the real MM stream starts ----
            NWARM = 20
            dmy_ps = ps_pool.tile([128, 512], fp32, tag="ps", name="dmy_ps")
            for i in range(NWARM):
                nc.tensor.matmul(
                    dmy_ps[:],
                    dmy_sb[:, 0:128],
                    dmy_sb[:],
                    start=(i == 0),
                    stop=(i == NWARM - 1),
                )

            # ---- t-phase fp8: t = Af8 @ x8 -> psum [er, t] ----
            pt_tiles = []
            for tb in range(2):
                pt = ps_pool.tile([128, 512], fp32, tag="ps", name=f"pt{tb}")
                pt_tiles.append(pt)
                for kt in range(NP8):
                    nc.tensor.matmul(
                        pt[:],
                        af8_sb[:, 2 * kt:2 * kt + 2, :],
                        x8_sb[:, 2 * kt:2 * kt + 2, tb * 512:tb * 512 + 512],
                        start=(kt == 0),
                        stop=False,
                        perf_mode=DR,
                    )

            # ---- interleaved: main-loop fp8 parts for ob=0, tt=0..5 ----
            po_tiles = [None] * NTT

            def main_fp8(ob, tt):
                po = ps_pool.tile([128, 512], fp32, tag="ps", name=f"po_{ob}_{tt}")
                po_tiles[tt] = po
                for kt in range(NP8):
                    nc.tensor.matmul(
                        po[:],
                        x8_sb[:, 2 * kt:2 * kt + 2, tt * 128:tt * 128 + 128],
                        w8_tiles[ob][:, 2 * kt:2 * kt + 2, :],
                        start=(kt == 0),
                        stop=False,
                        perf_mode=DR,
                    )

            def main_rest(ob, tt):
                po = po_tiles[tt]
                for kb in range(NB):
                    nc.tensor.matmul(
                        po[:],
                        xb_sb[:, kb:kb + 1, tt * 128:tt * 128 + 128],
                        wb_tiles[ob][:, kb:kb + 1, :],
                        start=False,
                        stop=False,
                    )
                nc.tensor.matmul(
                    po[:],
                    u_sb[:, tt * 128:(tt + 1) * 128],
                    bfT_sb[:, ob * 512:(ob + 1) * 512],
                    start=False,
                    stop=True,
                )
                ot = otpool.tile([128, 512], bf16, tag="ot", name=f"ot_{ob}_{tt}")
                nc.vector.tensor_add(
                    ot[:], po[:], biasrep_sb[:, ob * 512:(ob + 1) * 512]
                )
                nc.sync.dma_start(
                    out[tt * 128:(tt + 1) * 128, ob * 512:(ob + 1) * 512],
                    ot[:],
                )

            for tt in range(6):
                main_fp8(0, tt)

            # ---- t-phase bf16 (per-chunk, both halves), then u = t * routing ----
            for kb in range(NB):
                for tb in range(2):
                    nc.tensor.matmul(
                        pt_tiles[tb][:],
                        afb_sb[:, kb:kb + 1, :],
                        xb_sb[:, kb:kb + 1, tb * 512:tb * 512 + 512],
                        start=False,
                        stop=(kb == NB - 1),
                    )
            for tb in range(2):
                nc.vector.tensor_scalar_mul(
                    u_sb[:, tb * 512:(tb + 1) * 512],
                    pt_tiles[tb][:],
                    svec_sb[:, 0:1],
                )

            # ---- main loop ----
            for ob in range(NOB):
                if w8_tiles[ob] is None:
                    load_w8(ob)
                if wb_tiles[ob] is None:
                    load_wb(ob)
                if ob == 0:
                    for tt in range(6):
                        main_rest(0, tt)
                    for tt in range(6, NTT):
                        main_fp8(0, tt)
                        main_rest(0, tt)
                else:
                    for tt in range(NTT):
                        main_fp8(ob, tt)
                        main_rest(ob, tt)

    nc.compile()
    return nc


def _q8(v):
    return np.clip(v, -240.0, 240.0).astype(FP8)


def _host_prep(x, W, b, A, B, router_W, router_b):
    f32 = np.float32
    XT = np.ascontiguousarray(x, dtype=f32).reshape(TOK, D).T  # [D, TOK]
    # fp8 rows [0 : N8*128), bf16 rows [N8*128 : D); chunk c row = c*128+p
    x8_all = np.ascontiguousarray(
        _q8(XT[:N8 * 128] * SX).reshape(N8, 128, TOK).transpose(1, 0, 2)
    )
    xb_all = np.ascontiguousarray(
        (XT[N8 * 128:] * SX).astype(BF16).reshape(NB, 128, TOK).transpose(1, 0, 2)
    )

    WT = np.asarray(W, f32).T  # [D, O]
    w8_h = np.ascontiguousarray(
        _q8(WT[:N8 * 128] * SW)
        .reshape(N8, 128, NOB, 512).transpose(2, 1, 0, 3)
    ).reshape(NOB * 128, N8, 512)
    wb_h = np.ascontiguousarray(
        (WT[N8 * 128:] * SW).astype(BF16)
        .reshape(NB, 128, NOB, 512).transpose(2, 1, 0, 3)
    ).reshape(NOB * 128, NB, 512)

    AfT = np.asarray(A, f32).reshape(ER, D).T  # [D, ER]
    af8_h = np.ascontiguousarray(
        _q8(AfT[:N8 * 128] * SW).reshape(N8, 128, ER).transpose(1, 0, 2)
    )
    afb_h = np.ascontiguousarray(
        (AfT[N8 * 128:] * SW).astype(BF16).reshape(NB, 128, ER).transpose(1, 0, 2)
    )

    bfT_h = (2.0 * np.transpose(B, (0, 2, 1)).reshape(ER, O)).astype(BF16)
    bias_h = np.ascontiguousarray(
        np.broadcast_to((np.asarray(b, f32) * (SX * SW)).astype(BF16)[None, :], (128, O))
    )

    # router on host (numpy, float64 — exact vs device quantization noise)
    xq = np.asarray(x, np.float64)[:, Q_LO:Q_HI, :]
    q = xq.mean(axis=1)
    logits = q @ np.asarray(router_W, np.float64).T + np.asarray(router_b, np.float64)
    ex = np.exp(logits - logits.max(-1, keepdims=True))
    routing = ex / ex.sum(-1, keepdims=True)          # [B, E]

    in_maps = []
    for c in range(NCORES):
        sv = np.repeat(routing[c // 2].astype(f32), R).reshape(128, 1)
        in_maps.append({
            "x8": np.ascontiguousarray(x8_all[:, :, c * TPC:(c + 1) * TPC]),
            "xb": np.ascontiguousarray(xb_all[:, :, c * TPC:(c + 1) * TPC]),
            "w8": w8_h,
            "wb": wb_h,
            "af8": af8_h,
            "afb": afb_h,
            "bfT": bfT_h,
            "biasrep": bias_h,
            "svec": np.ascontiguousarray(sv),
        })
    return in_maps


def kernel(x, W, b, A, B, router_W, router_b):
    global LAST_RESULTS
    from concourse.bass_utils import run_bass_kernel_spmd

    if "nc" not in _CACHE:
        _CACHE["nc"] = _build_nc()
    nc = _CACHE["nc"]

    in_maps = _host_prep(x, W, b, A, B, router_W, router_b)

    kwargs = {}
    if TRACE:
        kwargs.update(trace=True, trace_cores=list(range(NCORES)))
    res = run_bass_kernel_spmd(nc, in_maps, core_ids=list(range(NCORES)), **kwargs)
    LAST_RESULTS = res

    shards = [
        np.asarray(res.results[c]["out"]).astype(np.float32) for c in range(NCORES)
    ]
    full = np.concatenate(shards, axis=0) * OSCALE
    return full.reshape(B_, S, O).astype(np.float32)
